# revision 2
# baseline (speedup 1.0000x reference)
"""CenterLoss Trainium2 kernel.

loss = mean_b clip(||x_b - centers[labels_b]||^2, 1e-12, 1e12)

Shapes (hardcoded): x [8192, 512] f32, labels [8192] int64 in [0, 10000),
centers [10000, 512] f32.  Output: f32 scalar.

Strategy: data-parallel over batch across 8 cores (1024 rows each);
centers stay in HBM (replicated input) and each core gathers exactly the
1024 rows it needs.  The full [B, C] distmat of the reference is never
formed - only the diagonal entries distmat[b, labels_b] are needed, so
the kernel is memory-bound: ~4 MB of HBM traffic per core (2 MB x-shard
+ 2 MB gathered centers).

v2 over the indirect-DMA baseline: the gather uses dma_gather (int16
indices, descriptors generated by all 8 Q7 cores in parallel) in 2
instructions of 512 rows instead of 8 serial indirect DMAs; x is loaded
in 4 chunked DMAs so the DVE/ACT compute chases the DMA stream
tile-by-tile instead of waiting on one 2 MB transfer.

dma_gather places gathered row i at (partition i%128, slot i//128); the
x DMA keeps row r at (partition r//8, slot r%8) for a contiguous 16 KB
per-partition load.  The label order is permuted on the host so slot
(p, t) of both buffers refers to the same batch row - valid because the
output is a mean over rows, so row order is irrelevant.

Per tile: DVE subtract then ACT square with row-accumulate into
dist[p, t].  The [128, 8] per-row distances go back to the host, which
applies clip and the global mean.
"""

import sys

import numpy as np

try:
    import concourse  # noqa: F401
except ImportError:  # pragma: no cover
    sys.path.insert(0, "/opt/trn_rl_repo")

B, D, C = 8192, 512, 10000
N_CORES = 8
P = 128
ROWS = B // N_CORES  # 1024 rows per core
T = ROWS // P        # 8 tiles of 128 rows
NGATHER = 2          # gather instructions per core
GROWS = ROWS // NGATHER
XCHUNKS = 4          # x load split

CLAMP_MIN = 1e-12
CLAMP_MAX = 1e12

_CACHE = {}


def _build():
    import concourse.bacc as bacc
    import concourse.tile as tile
    from concourse import bass, mybir

    f32 = mybir.dt.float32
    i16 = mybir.dt.int16

    nc = bacc.Bacc("TRN2", target_bir_lowering=False, num_devices=N_CORES)
    x = nc.dram_tensor("x", [ROWS, D], f32, kind="ExternalInput")
    labels16 = nc.dram_tensor("labels16", [P, ROWS // 16], i16, kind="ExternalInput")
    centers = nc.dram_tensor("centers", [C, D], f32, kind="ExternalInput")
    out = nc.dram_tensor("out", [P, T], f32, kind="ExternalOutput")

    with tile.TileContext(nc) as tc:
        with (
            tc.tile_pool(name="big", bufs=1) as big,
            tc.tile_pool(name="small", bufs=1) as small,
            tc.tile_pool(name="work", bufs=4) as work,
        ):
            idx = small.tile([P, ROWS // 16], i16)
            dist = small.tile([P, T], f32)
            xbig = big.tile([P, T * D], f32)
            cbig = big.tile([P, T * D], f32)

            # idx[p, s]: wrapped dma_gather index layout, 128 B/partition.
            nc.sync.dma_start(out=idx[:], in_=labels16[:, :])
            # cbig[p, t*D:(t+1)*D] = centers[lab[t*128+p], :] in NGATHER
            # dma_gather calls; each emits GROWS 2 KB-row descriptors from
            # all 8 Q7 cores in parallel.
            for g in range(NGATHER):
                tpg = T // NGATHER
                nc.gpsimd.dma_gather(
                    cbig[:, g * tpg * D : (g + 1) * tpg * D].rearrange(
                        "p (t d) -> p t d", t=tpg
                    ),
                    centers[:, :],
                    idx[:, g * (GROWS // 16) : (g + 1) * (GROWS // 16)],
                    num_idxs=GROWS,
                    num_idxs_reg=GROWS,
                    elem_size=D,
                )
            # xbig[p, t*D:(t+1)*D] = x[p*T + t, :]; 16 KB contiguous per
            # partition, split into XCHUNKS DMAs for finer compute deps.
            xr = x[:, :].rearrange("(p t) d -> p (t d)", p=P)
            for c in range(XCHUNKS):
                sl = slice(c * (T // XCHUNKS) * D, (c + 1) * (T // XCHUNKS) * D)
                nc.sync.dma_start(out=xbig[:, sl], in_=xr[:, sl])
            for t in range(T):
                sl = slice(t * D, (t + 1) * D)
                diff = work.tile([P, D], f32, tag="diff")
                sq = work.tile([P, D], f32, tag="sq")
                nc.vector.tensor_sub(diff[:], xbig[:, sl], cbig[:, sl])
                # sq = diff^2 on ACT; dist[:, t] = row-sum(sq) via ACT accum.
                nc.scalar.activation(
                    sq[:],
                    diff[:],
                    mybir.ActivationFunctionType.Square,
                    accum_out=dist[:, t : t + 1],
                )
            nc.sync.dma_start(out=out[:, :], in_=dist[:])

    nc.compile()
    return nc


def get_nc():
    nc = _CACHE.get("nc")
    if nc is None:
        nc = _CACHE["nc"] = _build()
    return nc


def make_labels16(lab):
    """Pack a 1024-label shard into dma_gather's wrapped int16 layout.

    Gathered row i of gather-call g lands at (partition i%128, slot
    g*(T//NGATHER) + i//128); x row r sits at (partition r//T, slot r%T).
    Permute so both agree, then wrap: index i of call g is read from
    (partition i%16, column g*(GROWS//16) + i//16), replicated to all 8
    16-partition blocks (one per Q7 core).
    """
    lab = np.asarray(lab).astype(np.int16).reshape(P, T)
    cols = []
    tpg = T // NGATHER
    for g in range(NGATHER):
        perm = lab[:, g * tpg : (g + 1) * tpg].T.reshape(-1)  # [GROWS]
        cols.append(perm.reshape(GROWS // 16, 16).T)  # [16, GROWS//16]
    block = np.concatenate(cols, axis=1)  # [16, ROWS//16]
    return np.ascontiguousarray(np.tile(block, (8, 1)))  # [128, ROWS//16]


def make_in_maps(x, labels, centers):
    labels = np.asarray(labels)
    x = np.ascontiguousarray(x, dtype=np.float32)
    centers = np.ascontiguousarray(centers, dtype=np.float32)
    in_maps = []
    for i in range(N_CORES):
        lo, hi = i * ROWS, (i + 1) * ROWS
        in_maps.append(
            {
                "x": x[lo:hi],
                "labels16": make_labels16(labels[lo:hi]),
                "centers": centers,
            }
        )
    return in_maps


def finish(per_core_outs):
    """per_core_outs: list of 8 [P, T] arrays -> f32 scalar loss."""
    d = np.concatenate([np.asarray(o).reshape(-1) for o in per_core_outs])
    d = np.clip(d, CLAMP_MIN, CLAMP_MAX)
    return np.asarray(np.mean(d, dtype=np.float64), dtype=np.float32)


def kernel(x, labels, centers):
    from concourse.bass_utils import run_bass_kernel_spmd

    nc = get_nc()
    in_maps = make_in_maps(x, labels, centers)
    res = run_bass_kernel_spmd(nc, in_maps, core_ids=list(range(N_CORES)))
    return finish([r["out"] for r in res.results])
